# revision 20
# baseline (speedup 1.0000x reference)
"""Trainium2 Bass kernel for nn_DGODE (graph ODE over utterance nodes).

Self-contained: hardcodes all shapes. Strategy:
- Row-shard B=4096 nodes over 8 cores (512 rows each).
- The unnormalized adjacency S is symmetric and exp(-0.1|i-j|) decays so
  fast that entries with |i-j| > 128 are negligible relative to the row
  sum: each core builds only a banded window S[jwin, rows_c] (jwin = own
  rows +-128) directly in transposed orientation, with the row
  normalization folded in. It stays SBUF-resident for all 16 RK4 evals.
- Per ODE eval, the collective carries only each core's k = ode_func
  output edge rows (AllGather of [256,128]); each core maintains a
  row-form window of the RK4 base state h and assembles the next y
  window locally, so the banded matmul and first MLP matmul of the next
  eval overlap the collective.
- MLP entirely in transposed orientation so every matmul has N=512;
  matmuls in float32r (full-rate fp32, ~1e-4 precision).

Row-form window layout (chunks of 128 rows): own rows stored as
[edge0 (rows 0:128), edge1 (rows 384:512)] + [int0 (128:256), int1
(256:384)] so the collective input (the two edges) is ready after only
two PE transposes.
"""

import sys

if "/opt/trn_rl_repo" not in sys.path:
    sys.path.insert(0, "/opt/trn_rl_repo")

import numpy as np

import concourse.bacc as bacc
import concourse.bass as bass
import concourse.mybir as mybir
import concourse.tile as tile
from concourse.bass_utils import run_bass_kernel_spmd

F32 = mybir.dt.float32
F32R = mybir.dt.float32r
U32 = mybir.dt.uint32
AF = mybir.ActivationFunctionType
ALU = mybir.AluOpType

NCORES = 8
B = 4096
D_IN = 1856
D_PAD = 1920           # 15 * 128
ND = D_PAD // 128
H = 128
R = B // NCORES        # 512 rows per core
P = 128
NW = 6                 # window chunks: [halo_l | own x4 | halo_r]
WIN = NW * P           # 768-row banded window (W = 128)
N_STEPS = 4
DT = 1.0 / N_STEPS
A1, A2, BETA = 0.8, 0.5, 0.1
SENT = -3.0e7          # sentinel j for out-of-range window rows -> S = 0

# own-chunk storage order [e0, i0, i1, e1] -> window chunk index 1 + t
OWN_EDGE = (0, 3)      # window own chunks held in the "edge" tiles
OWN_INT = (1, 2)

_CACHED_NC = None


def build_nc():
    nc = bacc.Bacc(
        "TRN2",
        target_bir_lowering=False,
        debug=False,
        enable_asserts=True,
        num_devices=NCORES,
    )

    # ---- per-core external inputs ----
    xT_d = nc.dram_tensor("xT", [D_PAD, R], F32R, kind="ExternalInput")
    xTh_d = nc.dram_tensor("xTh", [D_PAD, 2 * P], F32R, kind="ExternalInput")
    wp_d = nc.dram_tensor("wp", [D_PAD, H], F32R, kind="ExternalInput")
    bp_d = nc.dram_tensor("bp", [H, 1], F32, kind="ExternalInput")
    w1_d = nc.dram_tensor("w1", [2 * H, H], F32R, kind="ExternalInput")
    b1_d = nc.dram_tensor("b1", [H, 1], F32, kind="ExternalInput")
    w2_d = nc.dram_tensor("w2", [H, H], F32R, kind="ExternalInput")
    b2_d = nc.dram_tensor("b2", [H, 1], F32, kind="ExternalInput")
    ident_d = nc.dram_tensor("ident", [P, P], F32, kind="ExternalInput")
    iidx_d = nc.dram_tensor("iidx", [P, R], F32, kind="ExternalInput")
    spki_d = nc.dram_tensor("spki", [P, R], F32, kind="ExternalInput")
    ai_d = nc.dram_tensor("ai", [P, R], F32, kind="ExternalInput")
    bi_d = nc.dram_tensor("bi", [P, R], F32, kind="ExternalInput")
    ci_d = nc.dram_tensor("ci", [P, R], F32, kind="ExternalInput")
    svi_d = nc.dram_tensor("svi", [P, R], F32, kind="ExternalInput")
    njw_d = nc.dram_tensor("njw", [P, NW], F32, kind="ExternalInput")
    jw_d = nc.dram_tensor("jw", [P, NW], F32, kind="ExternalInput")
    spkj_d = nc.dram_tensor("spkj", [P, NW], F32, kind="ExternalInput")
    aj_d = nc.dram_tensor("aj", [P, NW], F32, kind="ExternalInput")
    bj_d = nc.dram_tensor("bj", [P, NW], F32, kind="ExternalInput")
    cj_d = nc.dram_tensor("cj", [P, NW], F32, kind="ExternalInput")
    svj_d = nc.dram_tensor("svj", [P, NW], F32, kind="ExternalInput")
    hidx_d = nc.dram_tensor("hidx", [1, 2], U32, kind="ExternalInput")

    out_d = nc.dram_tensor("hT_out", [H, R], F32, kind="ExternalOutput")

    with tile.TileContext(nc) as tc:
        with (
            tc.tile_pool(name="consts", bufs=1) as cs,
            tc.tile_pool(name="work", bufs=2) as wk,
            tc.tile_pool(name="states", bufs=1) as st,
            tc.tile_pool(name="ps", bufs=3, space="PSUM") as ps,
            tc.tile_pool(name="pst", bufs=2, space="PSUM") as pst,
            tc.tile_pool(name="dram", bufs=1, space="DRAM") as dram,
        ):
            # ---------- collective warm-up (overlaps the setup phase) ----------
            warm_f = cs.tile([1, P], F32, tag="warm")
            nc.vector.memset(warm_f[:], 0.0)
            warm_in = dram.tile([1, P], F32, tag="warm_in")
            warm_out = dram.tile([NCORES, P], F32, tag="warm_out",
                                 addr_space="Shared")
            nc.gpsimd.dma_start(warm_in[:], warm_f[:])
            nc.gpsimd.collective_compute(
                "AllGather",
                ALU.bypass,
                replica_groups=[list(range(NCORES))],
                ins=[warm_in[:].opt()],
                outs=[warm_out[:].opt()],
            )

            # ---------- load constants ----------
            def load_const(dram_t, shape, name, rdtype=None):
                t = cs.tile(shape, F32, tag=name)
                nc.sync.dma_start(t[:], dram_t[:])
                if rdtype is None:
                    return t
                tr = cs.tile(shape, rdtype, tag=name + "_r")
                nc.vector.tensor_copy(tr[:], t[:])
                return tr

            wp_r = cs.tile([P, ND, H], F32R, tag="wp_r")
            nc.sync.dma_start(wp_r[:], wp_d[:].rearrange("(n p) m -> p n m", p=P))
            w1_r = cs.tile([P, 2, H], F32R, tag="w1_r")
            nc.sync.dma_start(w1_r[:], w1_d[:].rearrange("(n p) m -> p n m", p=P))
            w2_r = cs.tile([H, H], F32R, tag="w2_r")
            nc.sync.dma_start(w2_r[:], w2_d[:])
            ident = load_const(ident_d, [P, P], "ident")
            bp_c = load_const(bp_d, [H, 1], "bp")
            b1_c = load_const(b1_d, [H, 1], "b1")
            b2_c = load_const(b2_d, [H, 1], "b2")

            iidx = load_const(iidx_d, [P, R], "iidx")
            spki = load_const(spki_d, [P, R], "spki")
            ai = load_const(ai_d, [P, R], "ai")
            bi = load_const(bi_d, [P, R], "bi")
            ci = load_const(ci_d, [P, R], "ci")
            svi = load_const(svi_d, [P, R], "svi")
            njw = load_const(njw_d, [P, NW], "njw")
            jw = load_const(jw_d, [P, NW], "jw")
            spkj = load_const(spkj_d, [P, NW], "spkj")
            aj = load_const(aj_d, [P, NW], "aj")
            bj = load_const(bj_d, [P, NW], "bj")
            cj = load_const(cj_d, [P, NW], "cj")
            svj = load_const(svj_d, [P, NW], "svj")

            hidx_sb = cs.tile([1, 2], U32, tag="hidx")
            nc.sync.dma_start(hidx_sb[:], hidx_d[:])
            regs_l = nc.alloc_registers("hl_reg")
            nc.regs_load(regs_l, hidx_sb[0:1, 0:1])
            hl_v = nc.snap(regs_l, donate=True)
            regs_r = nc.alloc_registers("hr_reg")
            nc.regs_load(regs_r, hidx_sb[0:1, 1:2])
            hr_v = nc.snap(regs_r, donate=True)

            # ---------- input projection: hT = (X @ Wp).T + bp ----------
            # halo columns projected locally too (features for own rows
            # +-128 are fed per-core), so no init collective is needed.
            with tc.tile_pool(name="xtp", bufs=3) as xtp:
                xv = xT_d[:].rearrange("(n p) m -> p n m", p=P)
                xhv = xTh_d[:].rearrange("(n p) m -> p n m", p=P)
                h0_ps = ps.tile([P, R], F32, tag="ps")
                h0h_ps4 = pst.tile([P, 4, P], F32, tag="z2e")
                h0h_ps = h0h_ps4[:, 0:2, :]
                for d in range(ND):
                    xd = xtp.tile([P, R], F32R, tag="xd")
                    nc.sync.dma_start(xd[:], xv[:, d, :])
                    nc.tensor.matmul(
                        h0_ps[:], wp_r[:, d, :], xd[:],
                        start=(d == 0), stop=(d == ND - 1),
                    )
                    xhd = xtp.tile([P, 2 * P], F32R, tag="xhd")
                    nc.scalar.dma_start(xhd[:], xhv[:, d, :])
                    nc.tensor.matmul(
                        h0h_ps.rearrange("p a b -> p (a b)"),
                        wp_r[:, d, :], xhd[:],
                        start=(d == 0), stop=(d == ND - 1),
                    )
            hT = st.tile([P, R], F32, tag="hT")
            nc.scalar.activation(hT[:], h0_ps[:], AF.Identity, bias=bp_c[:], scale=1.0)
            hT_r = st.tile([P, R], F32R, tag="hT_r")
            nc.scalar.activation(hT_r[:], h0_ps[:], AF.Identity, bias=bp_c[:], scale=1.0)
            hTh = st.tile([P, 2, P], F32, tag="hTh")
            nc.scalar.activation(hTh[:], h0h_ps, AF.Identity, bias=bp_c[:],
                                 scale=1.0)

            # ---------- banded adjacency build (transposed, unnormalized) ----------
            s_tiles = []
            for k in range(NW):
                adt = wk.tile([P, R], F32, tag="adt")
                nc.scalar.activation(adt[:], iidx[:], AF.Abs,
                                     bias=njw[:, k : k + 1], scale=1.0)
                Tt = wk.tile([P, R], F32, tag="Tt")
                nc.scalar.activation(Tt[:], adt[:], AF.Exp, scale=-BETA)

                Pm = wk.tile([P, R], F32, tag="Pm")
                nc.vector.tensor_scalar(Pm[:], spki[:], spkj[:, k : k + 1], None,
                                        ALU.is_equal)
                m1 = wk.tile([P, R], F32, tag="m1")
                nc.vector.tensor_scalar(m1[:], ai[:], aj[:, k : k + 1], None, ALU.min)
                m2 = wk.tile([P, R], F32, tag="m2")
                nc.vector.scalar_tensor_tensor(m2[:], bi[:], bj[:, k : k + 1], m1[:],
                                               ALU.min, ALU.add)
                m3 = wk.tile([P, R], F32, tag="m3")
                nc.vector.scalar_tensor_tensor(m3[:], ci[:], cj[:, k : k + 1], m2[:],
                                               ALU.min, ALU.add)
                u0 = wk.tile([P, R], F32, tag="u0")
                nc.vector.scalar_tensor_tensor(u0[:], m3[:], 2.0 * A2 / 3.0, svi[:],
                                               ALU.mult, ALU.add)
                uu = wk.tile([P, R], F32, tag="uu")
                nc.vector.tensor_scalar(uu[:], u0[:], svj[:, k : k + 1], None,
                                        ALU.subtract)
                t1 = wk.tile([P, R], F32, tag="t1")
                nc.vector.tensor_scalar(t1[:], uu[:], -1.0, A1, ALU.mult, ALU.add)
                t2 = wk.tile([P, R], F32, tag="t2")
                nc.gpsimd.tensor_tensor(t2[:], Pm[:], t1[:], ALU.mult)
                qq = wk.tile([P, R], F32, tag="qq")
                nc.vector.tensor_tensor(qq[:], uu[:], t2[:], ALU.add)
                s0 = wk.tile([P, R], F32, tag="s0")
                nc.vector.tensor_tensor(s0[:], Tt[:], qq[:], ALU.mult)
                dm = wk.tile([P, R], F32, tag="dm")
                nc.vector.tensor_scalar(dm[:], iidx[:], jw[:, k : k + 1], 1.0 - A1,
                                        ALU.is_equal, ALU.mult)
                sk = cs.tile([P, R], F32R, tag=f"sk{k}")
                nc.vector.tensor_tensor(sk[:], s0[:], dm[:], ALU.add)
                s_tiles.append(sk)

            # ---------- row sums d_i, reciprocal, fold into S ----------
            ones_f = cs.tile([P, 1], F32, tag="ones_f")
            nc.vector.memset(ones_f[:], 1.0)
            ones_r = cs.tile([P, 1], F32R, tag="ones")
            nc.vector.tensor_copy(ones_r[:], ones_f[:])
            d_ps = ps.tile([P, R], F32, tag="ps")
            for k in range(NW):
                nc.tensor.matmul(d_ps[0:1, :], ones_r[:], s_tiles[k][:],
                                 start=(k == 0), stop=(k == NW - 1))
            dsum = cs.tile([1, R], F32, tag="dsum")
            nc.vector.tensor_scalar(dsum[:], d_ps[0:1, :], 1e-8, None, ALU.add)
            rd = cs.tile([1, R], F32R, tag="rd")
            with nc.allow_low_precision(reason="f32r is full-width storage"):
                nc.vector.reciprocal(rd[:], dsum[:])
            onesrow_f = cs.tile([1, P], F32, tag="onesrow_f")
            nc.vector.memset(onesrow_f[:], 1.0)
            onesrow_r = cs.tile([1, P], F32R, tag="onesrow")
            nc.vector.tensor_copy(onesrow_r[:], onesrow_f[:])
            rdb_ps = ps.tile([P, R], F32, tag="ps")
            nc.tensor.matmul(rdb_ps[:], onesrow_r[:], rd[:])
            for k in range(NW):
                nc.vector.tensor_tensor(s_tiles[k][:], s_tiles[k][:], rdb_ps[:],
                                        ALU.mult)

            # s_tiles for the own chunks in [e0, i0, i1, e1] order, halos:
            s_own = [s_tiles[1], s_tiles[2], s_tiles[3], s_tiles[4]]
            s_own_order = [0, 3, 1, 2]      # issue edges' MMs first
            s_halo = [s_tiles[0], s_tiles[5]]
            # RK4-coefficient-prescaled halo S so the post-fetch halo matmuls
            # consume the raw gathered k directly (no y_h build on the path).
            sh_half, sh_full = [], []
            for n in range(2):
                t_h = cs.tile([P, R], F32R, tag=f"shh{n}")
                nc.vector.tensor_scalar(t_h[:], s_halo[n][:], 0.5 * DT, None,
                                        ALU.mult)
                sh_half.append(t_h)
                t_f = cs.tile([P, R], F32R, tag=f"shf{n}")
                nc.vector.tensor_scalar(t_f[:], s_halo[n][:], DT, None,
                                        ALU.mult)
                sh_full.append(t_f)

            ag_in = dram.tile([2 * P, H], F32R, tag="ag_in")

            def transpose_pair(srcT, chunks, tag, dtype=F32):
                """Transpose two [128,128] column blocks of a T-form tile
                into a row-form [P,2,P] tile."""
                tp = pst.tile([P, 2, P], F32, tag="tp")
                for n, t in enumerate(chunks):
                    nc.tensor.transpose(tp[:, n, :], srcT[:, t * P : (t + 1) * P],
                                        ident[:])
                row = wk.tile([P, 2, P], dtype, tag=tag)
                nc.vector.tensor_copy(row[:], tp[:])
                return row

            def send_edges(edge_row):
                nc.sync.dma_start(
                    ag_in[:].bitcast(F32).rearrange("(n p) m -> p n m", p=P),
                    edge_row[:],
                )

            def do_ag(it, tag="halo"):
                ag_out = dram.tile([NCORES * 2 * P, H], F32R, tag=f"ago{it}",
                                   addr_space="Shared")
                nc.gpsimd.collective_compute(
                    "AllGather",
                    ALU.bypass,
                    replica_groups=[list(range(NCORES))],
                    ins=[ag_in[:].opt()],
                    outs=[ag_out[:].opt()],
                )
                agv = ag_out[:].rearrange("(n p) m -> p n m", p=P)
                halo = wk.tile([P, 2, P], F32R, tag=tag)
                nc.scalar.dma_start(halo[:, 0:1, :], agv[:, bass.ds(hl_v, 1), :])
                nc.sync.dma_start(halo[:, 1:2, :], agv[:, bass.ds(hr_v, 1), :])
                return halo

            # ---------- init: h row-form window (halo computed locally) ----------
            h_e = transpose_pair(hT, OWN_EDGE, "h_e")     # own edge rows of h
            h_i = transpose_pair(hT, OWN_INT, "h_i")      # own interior rows
            h_h = transpose_pair(hTh[:].rearrange("p a b -> p (a b)"), (0, 1),
                                 "h_h0", dtype=F32R)       # halo rows of h

            def to_r(src, tag):
                r = wk.tile([P, 2, P], F32R, tag=tag)
                nc.vector.tensor_copy(r[:], src[:])
                return r

            y_e, y_i = to_r(h_e, "y_e"), to_r(h_i, "y_i")
            y_T = hT_r

            # row-form accumulators for h window update
            acc_e = st.tile([P, 2, P], F32, tag="acc_e")
            acc_i = st.tile([P, 2, P], F32, tag="acc_i")
            acc_h = st.tile([P, 2, P], F32, tag="acc_h")
            accT = st.tile([P, R], F32, tag="accT")

            # ---------- RK4 loop: 16 ODE evaluations ----------
            k_h = None
            for it in range(16):
                sub = it % 4
                last = it == 15

                # ode_func: hn = S_own^T y_own + S_halo^T h_h (+ c*S_halo^T k_h)
                hn_ps = ps.tile([P, R], F32, tag="ps")
                for n, t in enumerate(s_own_order):
                    src = y_e if t in OWN_EDGE else y_i
                    idx = OWN_EDGE.index(t) if t in OWN_EDGE else OWN_INT.index(t)
                    nc.tensor.matmul(hn_ps[:], src[:, idx, :], s_own[t][:],
                                     start=(n == 0), stop=False)
                z1_ps = ps.tile([P, R], F32, tag="ps")
                nc.tensor.matmul(z1_ps[:], w1_r[:, 0, :], y_T[:],
                                 start=True, stop=False)
                for n in range(2):
                    nc.tensor.matmul(hn_ps[:], h_h[:, n, :], s_halo[n][:],
                                     start=False, stop=(sub == 0 and n == 1))
                if sub != 0:
                    sh = sh_full if sub == 3 else sh_half
                    for n in range(2):
                        nc.tensor.matmul(hn_ps[:], k_h[:, n, :], sh[n][:],
                                         start=False, stop=(n == 1))
                hn_r = wk.tile([P, R], F32R, tag="hn_r")
                nc.scalar.activation(hn_r[:], hn_ps[:], AF.Copy, bias=0.0, scale=1.0)
                nc.tensor.matmul(z1_ps[:], w1_r[:, 1, :], hn_r[:],
                                 start=False, stop=True)
                th_r = wk.tile([P, R], F32R, tag="th_r")
                nc.scalar.activation(th_r[:], z1_ps[:], AF.Tanh, bias=b1_c[:],
                                     scale=1.0)
                # edge-first z2: edge columns of kt come off the critical path
                # to the collective sooner; interior follows.
                z2_ps4 = pst.tile([P, 4, P], F32, tag="z2e")
                nc.tensor.matmul(z2_ps4[:, 0, :], w2_r[:], th_r[:, 0:P])
                nc.tensor.matmul(z2_ps4[:, 1, :], w2_r[:], th_r[:, 3 * P : 4 * P])
                kt = wk.tile([P, R], F32, tag="kt", bufs=3)
                nc.vector.tensor_scalar(kt[:, 0:P], z2_ps4[:, 0, :], b2_c[:],
                                        None, ALU.add)
                nc.vector.tensor_scalar(kt[:, 3 * P : 4 * P], z2_ps4[:, 1, :],
                                        b2_c[:], None, ALU.add)

                def finish_kt():
                    z2i = z2_ps4[:, 2:4, :]
                    nc.tensor.matmul(z2i.rearrange("p a b -> p (a b)"),
                                     w2_r[:], th_r[:, P : 3 * P])
                    nc.scalar.activation(kt[:, P : 3 * P],
                                         z2i.rearrange("p a b -> p (a b)"),
                                         AF.Identity, bias=b2_c[:], scale=1.0)

                if last:
                    finish_kt()
                    acc4 = wk.tile([P, R], F32, tag="acc4")
                    nc.vector.tensor_tensor(acc4[:], accT[:], kt[:], ALU.add)
                    hT_fin = st.tile([P, R], F32, tag="hT_fin")
                    nc.vector.scalar_tensor_tensor(hT_fin[:], acc4[:], DT / 6.0,
                                                   hT[:], ALU.mult, ALU.add)
                    nc.sync.dma_start(out_d[:], hT_fin[:])
                    break

                # edge k rows -> collective (critical path)
                k_e = transpose_pair(kt, OWN_EDGE, "k_e")
                send_edges(k_e)
                k_h = do_ag(it)
                finish_kt()
                k_i = transpose_pair(kt, OWN_INT, "k_i")

                # T-form accumulator + next-y (overlap the collective)
                if sub == 0:
                    nc.vector.tensor_copy(accT[:], kt[:])
                elif sub in (1, 2):
                    nc.vector.scalar_tensor_tensor(accT[:], kt[:], 2.0, accT[:],
                                                   ALU.mult, ALU.add)

                if sub < 3:
                    coef = 0.5 * DT if sub < 2 else DT
                    y_T = wk.tile([P, R], F32R, tag="y_T", bufs=3)
                    nc.vector.scalar_tensor_tensor(y_T[:], kt[:], coef, hT[:],
                                                   ALU.mult, ALU.add)
                    # row-form next-y window
                    y_e = wk.tile([P, 2, P], F32R, tag="y_e")
                    nc.vector.scalar_tensor_tensor(y_e[:], k_e[:], coef, h_e[:],
                                                   ALU.mult, ALU.add)
                    y_i = wk.tile([P, 2, P], F32R, tag="y_i")
                    nc.vector.scalar_tensor_tensor(y_i[:], k_i[:], coef, h_i[:],
                                                   ALU.mult, ALU.add)
                    # row-form accumulators
                    if sub == 0:
                        for a, s in ((acc_e, k_e), (acc_i, k_i), (acc_h, k_h)):
                            nc.vector.tensor_copy(a[:], s[:])
                    else:
                        for a, s in ((acc_e, k_e), (acc_i, k_i), (acc_h, k_h)):
                            nc.vector.scalar_tensor_tensor(a[:], s[:], 2.0, a[:],
                                                           ALU.mult, ALU.add)
                else:
                    # step boundary: h' = h + dt/6 (acc + k4), rebuild windows
                    acc4 = wk.tile([P, R], F32, tag="acc4")
                    nc.vector.tensor_tensor(acc4[:], accT[:], kt[:], ALU.add)
                    hT_new = st.tile([P, R], F32, tag=f"hT{it}")
                    nc.vector.scalar_tensor_tensor(hT_new[:], acc4[:], DT / 6.0,
                                                   hT[:], ALU.mult, ALU.add)
                    hT = hT_new
                    hT_r = st.tile([P, R], F32R, tag=f"hTr{it}")
                    nc.vector.tensor_copy(hT_r[:], hT[:])
                    y_T = hT_r

                    new_h = []
                    for nm, a, s, h_old in (("e", acc_e, k_e, h_e),
                                            ("i", acc_i, k_i, h_i),
                                            ("h", acc_h, k_h, h_h)):
                        a4 = wk.tile([P, 2, P], F32, tag=f"a4{nm}")
                        nc.vector.tensor_tensor(a4[:], a[:], s[:], ALU.add)
                        hn_new = st.tile([P, 2, P], F32R if nm == "h" else F32,
                                         tag=f"h_{nm}{it}")
                        nc.vector.scalar_tensor_tensor(hn_new[:], a4[:], DT / 6.0,
                                                       h_old[:], ALU.mult, ALU.add)
                        new_h.append(hn_new)
                    h_e, h_i, h_h = new_h
                    y_e, y_i = to_r(h_e, "y_e"), to_r(h_i, "y_i")

    nc.compile()
    return nc


def get_nc():
    global _CACHED_NC
    if _CACHED_NC is None:
        _CACHED_NC = build_nc()
    return _CACHED_NC


def prep_inputs(features, speaker_ids, modality_masks, Wp, bp, W1, b1, W2, b2):
    features = np.asarray(features, dtype=np.float32)
    spk = np.asarray(speaker_ids).astype(np.float32)
    mm = np.asarray(modality_masks, dtype=np.float32)
    Wp = np.asarray(Wp, dtype=np.float32)
    bp = np.asarray(bp, dtype=np.float32)
    W1 = np.asarray(W1, dtype=np.float32)
    b1 = np.asarray(b1, dtype=np.float32)
    W2 = np.asarray(W2, dtype=np.float32)
    b2 = np.asarray(b2, dtype=np.float32)

    wp_pad = np.zeros((D_PAD, H), dtype=np.float32)
    wp_pad[:D_IN] = Wp
    s_all = mm.sum(axis=1)
    sv_all = (A2 - (A2 / 3.0) * s_all).astype(np.float32)
    svj_all = ((A2 / 3.0) * s_all).astype(np.float32)
    ident = np.eye(P, dtype=np.float32)

    def rep(v):
        return np.ascontiguousarray(np.broadcast_to(v, (P, v.shape[0])), dtype=np.float32)

    def pm(v):
        return np.ascontiguousarray(v.reshape(NW, P).T, dtype=np.float32)

    in_maps = []
    for c in range(NCORES):
        rows = slice(c * R, (c + 1) * R)
        rb = c * R - P
        jwin = np.arange(rb, rb + WIN)
        valid = (jwin >= 0) & (jwin < B)
        jcl = np.clip(jwin, 0, B - 1)
        jvals = np.where(valid, jwin.astype(np.float32), np.float32(SENT))
        xT = np.zeros((D_PAD, R), dtype=np.float32)
        xT[:D_IN] = features[rows].T
        xTh = np.zeros((D_PAD, 2 * P), dtype=np.float32)
        lo, hi = c * R - P, (c + 1) * R + P
        if lo >= 0:
            xTh[:D_IN, 0:P] = features[lo : c * R].T
        if hi <= B:
            xTh[:D_IN, P : 2 * P] = features[(c + 1) * R : hi].T
        ivals = np.arange(c * R, (c + 1) * R).astype(np.float32)
        hl_idx = 2 * (c - 1) + 1 if c > 0 else 0
        hr_idx = 2 * (c + 1) if c < NCORES - 1 else 0
        in_maps.append({
            "xT": xT,
            "xTh": xTh,
            "wp": wp_pad,
            "bp": bp.reshape(H, 1).copy(),
            "w1": W1.copy(),
            "b1": b1.reshape(H, 1).copy(),
            "w2": W2.copy(),
            "b2": b2.reshape(H, 1).copy(),
            "ident": ident,
            "iidx": rep(ivals),
            "spki": rep(spk[rows]),
            "ai": rep(mm[rows, 0]),
            "bi": rep(mm[rows, 1]),
            "ci": rep(mm[rows, 2]),
            "svi": rep(sv_all[rows]),
            "njw": pm(-jvals),
            "jw": pm(jvals),
            "spkj": pm(spk[jcl]),
            "aj": pm(mm[jcl, 0]),
            "bj": pm(mm[jcl, 1]),
            "cj": pm(mm[jcl, 2]),
            "svj": pm(svj_all[jcl]),
            "hidx": np.array([[hl_idx, hr_idx]], dtype=np.uint32),
        })
    return in_maps


def kernel(features, speaker_ids, modality_masks, Wp, bp, W1, b1, W2, b2,
           _runner=None):
    in_maps = prep_inputs(features, speaker_ids, modality_masks,
                          Wp, bp, W1, b1, W2, b2)
    nc = get_nc()
    if _runner is not None:
        results = _runner(nc, in_maps)
    else:
        results = run_bass_kernel_spmd(nc, in_maps, list(range(NCORES))).results
    out = np.concatenate([results[c]["hT_out"].T for c in range(NCORES)], axis=0)
    return np.ascontiguousarray(out, dtype=np.float32)



# revision 27
# speedup vs baseline: 1.3300x; 1.3300x over previous
"""Trainium2 Bass kernel for nn_DGODE (graph ODE over utterance nodes).

Self-contained: hardcodes all shapes. Strategy:
- Row-shard B=4096 nodes over 8 cores (512 rows each).
- The unnormalized adjacency S is symmetric and exp(-0.1|i-j|) decays so
  fast that entries with |i-j| > 128 are negligible relative to the row
  sum: each core builds only a banded window S[jwin, rows_c] (jwin = own
  rows +-128) directly in transposed orientation, with the row
  normalization folded in. It stays SBUF-resident for all 16 RK4 evals.
- Per ODE eval, the collective carries only each core's k = ode_func
  output edge rows (AllGather of [256,128]); each core maintains a
  row-form window of the RK4 base state h and assembles the next y
  window locally, so the banded matmul and first MLP matmul of the next
  eval overlap the collective.
- MLP entirely in transposed orientation so every matmul has N=512;
  matmuls in float32r (full-rate fp32, ~1e-4 precision).

Row-form window layout (chunks of 128 rows): own rows stored as
[edge0 (rows 0:128), edge1 (rows 384:512)] + [int0 (128:256), int1
(256:384)] so the collective input (the two edges) is ready after only
two PE transposes.
"""

import sys

if "/opt/trn_rl_repo" not in sys.path:
    sys.path.insert(0, "/opt/trn_rl_repo")

import numpy as np

import concourse.bacc as bacc
import concourse.bass as bass
import concourse.mybir as mybir
import concourse.tile as tile
from concourse.bass_utils import run_bass_kernel_spmd

F32 = mybir.dt.float32
F32R = mybir.dt.float32r
BF16 = mybir.dt.bfloat16
U32 = mybir.dt.uint32
AF = mybir.ActivationFunctionType
ALU = mybir.AluOpType

NCORES = 8
B = 4096
D_IN = 1856
D_PAD = 1920           # 15 * 128
ND = D_PAD // 128
H = 128
R = B // NCORES        # 512 rows per core
P = 128
NW = 6                 # window chunks: [halo_l | own x4 | halo_r]
WIN = NW * P           # 768-row banded window (W = 128)
N_STEPS = 4
DT = 1.0 / N_STEPS
A1, A2, BETA = 0.8, 0.5, 0.1
SENT = -3.0e7          # sentinel j for out-of-range window rows -> S = 0

# own-chunk storage order [e0, i0, i1, e1] -> window chunk index 1 + t
OWN_EDGE = (0, 3)      # window own chunks held in the "edge" tiles
OWN_INT = (1, 2)

_CACHED_NC = None


def build_nc():
    nc = bacc.Bacc(
        "TRN2",
        target_bir_lowering=False,
        debug=False,
        enable_asserts=True,
        num_devices=NCORES,
    )

    # ---- per-core external inputs ----
    xT_d = nc.dram_tensor("xT", [D_PAD, R], F32R, kind="ExternalInput")
    xTh_d = nc.dram_tensor("xTh", [D_PAD, 2 * P], F32R, kind="ExternalInput")
    wp_d = nc.dram_tensor("wp", [D_PAD, H], F32R, kind="ExternalInput")
    bp_d = nc.dram_tensor("bp", [H, 1], F32, kind="ExternalInput")
    w1_d = nc.dram_tensor("w1", [2 * H, H], F32R, kind="ExternalInput")
    b1_d = nc.dram_tensor("b1", [H, 1], F32, kind="ExternalInput")
    w2_d = nc.dram_tensor("w2", [H, H], F32R, kind="ExternalInput")
    b2_d = nc.dram_tensor("b2", [H, 1], F32, kind="ExternalInput")
    ident_d = nc.dram_tensor("ident", [P, P], F32, kind="ExternalInput")
    iidx_d = nc.dram_tensor("iidx", [P, R], F32, kind="ExternalInput")
    spki_d = nc.dram_tensor("spki", [P, R], F32, kind="ExternalInput")
    ai_d = nc.dram_tensor("ai", [P, R], F32, kind="ExternalInput")
    bi_d = nc.dram_tensor("bi", [P, R], F32, kind="ExternalInput")
    ci_d = nc.dram_tensor("ci", [P, R], F32, kind="ExternalInput")
    svi_d = nc.dram_tensor("svi", [P, R], F32, kind="ExternalInput")
    njw_d = nc.dram_tensor("njw", [P, NW], F32, kind="ExternalInput")
    jw_d = nc.dram_tensor("jw", [P, NW], F32, kind="ExternalInput")
    spkj_d = nc.dram_tensor("spkj", [P, NW], F32, kind="ExternalInput")
    aj_d = nc.dram_tensor("aj", [P, NW], F32, kind="ExternalInput")
    bj_d = nc.dram_tensor("bj", [P, NW], F32, kind="ExternalInput")
    cj_d = nc.dram_tensor("cj", [P, NW], F32, kind="ExternalInput")
    svj_d = nc.dram_tensor("svj", [P, NW], F32, kind="ExternalInput")
    hidx_d = nc.dram_tensor("hidx", [1, 2], U32, kind="ExternalInput")

    out_d = nc.dram_tensor("hT_out", [H, R], F32, kind="ExternalOutput")

    with tile.TileContext(nc) as tc:
        with (
            tc.tile_pool(name="consts", bufs=1) as cs,
            tc.tile_pool(name="work", bufs=2) as wk,
            tc.tile_pool(name="states", bufs=1) as st,
            tc.tile_pool(name="ps", bufs=3, space="PSUM") as ps,
            tc.tile_pool(name="pst", bufs=2, space="PSUM") as pst,
            tc.tile_pool(name="dram", bufs=1, space="DRAM") as dram,
        ):
            # ---------- collective warm-up (overlaps the setup phase) ----------
            warm_f = cs.tile([1, P], F32, tag="warm")
            nc.vector.memset(warm_f[:], 0.0)
            warm_in = dram.tile([1, P], F32, tag="warm_in")
            warm_out = dram.tile([NCORES, P], F32, tag="warm_out",
                                 addr_space="Shared")
            nc.gpsimd.dma_start(warm_in[:], warm_f[:])
            nc.gpsimd.collective_compute(
                "AllGather",
                ALU.bypass,
                replica_groups=[list(range(NCORES))],
                ins=[warm_in[:].opt()],
                outs=[warm_out[:].opt()],
            )

            # ---------- load constants ----------
            def load_const(dram_t, shape, name, rdtype=None):
                t = cs.tile(shape, F32, tag=name)
                nc.sync.dma_start(t[:], dram_t[:])
                if rdtype is None:
                    return t
                tr = cs.tile(shape, rdtype, tag=name + "_r")
                nc.vector.tensor_copy(tr[:], t[:])
                return tr

            wp_r = cs.tile([P, ND, H], F32R, tag="wp_r")
            nc.sync.dma_start(wp_r[:], wp_d[:].rearrange("(n p) m -> p n m", p=P))
            w1_r = cs.tile([P, 2, H], F32R, tag="w1_r")
            nc.sync.dma_start(w1_r[:], w1_d[:].rearrange("(n p) m -> p n m", p=P))
            w2_r = cs.tile([H, H], F32R, tag="w2_r")
            nc.sync.dma_start(w2_r[:], w2_d[:])
            ident = load_const(ident_d, [P, P], "ident")
            bp_c = load_const(bp_d, [H, 1], "bp")
            b1_c = load_const(b1_d, [H, 1], "b1")
            b2_c = load_const(b2_d, [H, 1], "b2")

            iidx = load_const(iidx_d, [P, R], "iidx")
            spki = load_const(spki_d, [P, R], "spki")
            ai = load_const(ai_d, [P, R], "ai")
            bi = load_const(bi_d, [P, R], "bi")
            ci = load_const(ci_d, [P, R], "ci")
            svi = load_const(svi_d, [P, R], "svi")
            njw = load_const(njw_d, [P, NW], "njw")
            jw = load_const(jw_d, [P, NW], "jw")
            spkj = load_const(spkj_d, [P, NW], "spkj")
            aj = load_const(aj_d, [P, NW], "aj")
            bj = load_const(bj_d, [P, NW], "bj")
            cj = load_const(cj_d, [P, NW], "cj")
            svj = load_const(svj_d, [P, NW], "svj")

            hidx_sb = cs.tile([1, 2], U32, tag="hidx")
            nc.sync.dma_start(hidx_sb[:], hidx_d[:])
            regs_l = nc.alloc_registers("hl_reg")
            nc.regs_load(regs_l, hidx_sb[0:1, 0:1])
            hl_v = nc.snap(regs_l, donate=True)
            regs_r = nc.alloc_registers("hr_reg")
            nc.regs_load(regs_r, hidx_sb[0:1, 1:2])
            hr_v = nc.snap(regs_r, donate=True)

            # ---------- input projection: hT = (X @ Wp).T + bp ----------
            # halo columns projected locally too (features for own rows
            # +-128 are fed per-core), so no init collective is needed.
            with tc.tile_pool(name="xtp", bufs=3) as xtp:
                xv = xT_d[:].rearrange("(n p) m -> p n m", p=P)
                xhv = xTh_d[:].rearrange("(n p) m -> p n m", p=P)
                h0_ps = ps.tile([P, R], F32, tag="ps")
                h0h_ps4 = pst.tile([P, 4, P], F32, tag="z2e")
                h0h_ps = h0h_ps4[:, 0:2, :]
                for d in range(ND):
                    xd = xtp.tile([P, R], F32R, tag="xd")
                    nc.sync.dma_start(xd[:], xv[:, d, :])
                    nc.tensor.matmul(
                        h0_ps[:], wp_r[:, d, :], xd[:],
                        start=(d == 0), stop=(d == ND - 1),
                    )
                    xhd = xtp.tile([P, 2 * P], F32R, tag="xhd")
                    nc.scalar.dma_start(xhd[:], xhv[:, d, :])
                    nc.tensor.matmul(
                        h0h_ps.rearrange("p a b -> p (a b)"),
                        wp_r[:, d, :], xhd[:],
                        start=(d == 0), stop=(d == ND - 1),
                    )
            hT = st.tile([P, R], F32, tag="hT")
            nc.scalar.activation(hT[:], h0_ps[:], AF.Identity, bias=bp_c[:], scale=1.0)
            hT_r = st.tile([P, R], F32R, tag="hT_r")
            nc.scalar.activation(hT_r[:], h0_ps[:], AF.Identity, bias=bp_c[:], scale=1.0)
            hTh = st.tile([P, 2, P], F32, tag="hTh")
            nc.scalar.activation(hTh[:], h0h_ps, AF.Identity, bias=bp_c[:],
                                 scale=1.0)

            # ---------- banded adjacency build (transposed, unnormalized) ----------
            s_tiles = []
            for k in range(NW):
                adt = wk.tile([P, R], F32, tag="adt")
                nc.scalar.activation(adt[:], iidx[:], AF.Abs,
                                     bias=njw[:, k : k + 1], scale=1.0)
                Tt = wk.tile([P, R], F32, tag="Tt")
                nc.scalar.activation(Tt[:], adt[:], AF.Exp, scale=-BETA)

                Pm = wk.tile([P, R], F32, tag="Pm")
                nc.vector.tensor_scalar(Pm[:], spki[:], spkj[:, k : k + 1], None,
                                        ALU.is_equal)
                m1 = wk.tile([P, R], F32, tag="m1")
                nc.vector.tensor_scalar(m1[:], ai[:], aj[:, k : k + 1], None, ALU.min)
                m2 = wk.tile([P, R], F32, tag="m2")
                nc.vector.scalar_tensor_tensor(m2[:], bi[:], bj[:, k : k + 1], m1[:],
                                               ALU.min, ALU.add)
                m3 = wk.tile([P, R], F32, tag="m3")
                nc.vector.scalar_tensor_tensor(m3[:], ci[:], cj[:, k : k + 1], m2[:],
                                               ALU.min, ALU.add)
                u0 = wk.tile([P, R], F32, tag="u0")
                nc.vector.scalar_tensor_tensor(u0[:], m3[:], 2.0 * A2 / 3.0, svi[:],
                                               ALU.mult, ALU.add)
                uu = wk.tile([P, R], F32, tag="uu")
                nc.vector.tensor_scalar(uu[:], u0[:], svj[:, k : k + 1], None,
                                        ALU.subtract)
                t1 = wk.tile([P, R], F32, tag="t1")
                nc.vector.tensor_scalar(t1[:], uu[:], -1.0, A1, ALU.mult, ALU.add)
                t2 = wk.tile([P, R], F32, tag="t2")
                nc.gpsimd.tensor_tensor(t2[:], Pm[:], t1[:], ALU.mult)
                qq = wk.tile([P, R], F32, tag="qq")
                nc.vector.tensor_tensor(qq[:], uu[:], t2[:], ALU.add)
                s0 = wk.tile([P, R], F32, tag="s0")
                nc.vector.tensor_tensor(s0[:], Tt[:], qq[:], ALU.mult)
                dm = wk.tile([P, R], F32, tag="dm")
                nc.vector.tensor_scalar(dm[:], iidx[:], jw[:, k : k + 1], 1.0 - A1,
                                        ALU.is_equal, ALU.mult)
                sk = cs.tile([P, R], F32R, tag=f"sk{k}")
                nc.vector.tensor_tensor(sk[:], s0[:], dm[:], ALU.add)
                s_tiles.append(sk)

            # ---------- row sums d_i, reciprocal, fold into S ----------
            ones_f = cs.tile([P, 1], F32, tag="ones_f")
            nc.vector.memset(ones_f[:], 1.0)
            ones_r = cs.tile([P, 1], F32R, tag="ones")
            nc.vector.tensor_copy(ones_r[:], ones_f[:])
            d_ps = ps.tile([P, R], F32, tag="ps")
            for k in range(NW):
                nc.tensor.matmul(d_ps[0:1, :], ones_r[:], s_tiles[k][:],
                                 start=(k == 0), stop=(k == NW - 1))
            dsum = cs.tile([1, R], F32, tag="dsum")
            nc.vector.tensor_scalar(dsum[:], d_ps[0:1, :], 1e-8, None, ALU.add)
            rd = cs.tile([1, R], F32R, tag="rd")
            with nc.allow_low_precision(reason="f32r is full-width storage"):
                nc.vector.reciprocal(rd[:], dsum[:])
            onesrow_f = cs.tile([1, P], F32, tag="onesrow_f")
            nc.vector.memset(onesrow_f[:], 1.0)
            onesrow_r = cs.tile([1, P], F32R, tag="onesrow")
            nc.vector.tensor_copy(onesrow_r[:], onesrow_f[:])
            rdb_ps = ps.tile([P, R], F32, tag="ps")
            nc.tensor.matmul(rdb_ps[:], onesrow_r[:], rd[:])
            for k in range(NW):
                nc.vector.tensor_tensor(s_tiles[k][:], s_tiles[k][:], rdb_ps[:],
                                        ALU.mult)

            # s_tiles for the own chunks in [e0, i0, i1, e1] order, halos:
            s_own = [s_tiles[1], s_tiles[2], s_tiles[3], s_tiles[4]]
            s_own_order = [0, 3, 1, 2]      # issue edges' MMs first
            s_halo = [s_tiles[0], s_tiles[5]]
            # RK4-coefficient-prescaled halo S so the post-fetch halo matmuls
            # consume the raw gathered k directly (no y_h build on the path).
            # bf16: the exchanged halo k is low-stakes (decayed S weights),
            # so the collective payload and its S multiplier run in bf16.
            sh_half, sh_full = [], []
            with nc.allow_low_precision(reason="bf16 halo-exchange payload"):
                for n in range(2):
                    t_h = cs.tile([P, R], BF16, tag=f"shh{n}")
                    nc.vector.tensor_scalar(t_h[:], s_halo[n][:], 0.5 * DT,
                                            None, ALU.mult)
                    sh_half.append(t_h)
                    t_f = cs.tile([P, R], BF16, tag=f"shf{n}")
                    nc.vector.tensor_scalar(t_f[:], s_halo[n][:], DT, None,
                                            ALU.mult)
                    sh_full.append(t_f)

            ag_in = dram.tile([2 * P, H], BF16, tag="ag_in")

            def transpose_pair(srcT, chunks, tag, dtype=F32):
                """Transpose two [128,128] column blocks of a T-form tile
                into a row-form [P,2,P] tile."""
                tp = pst.tile([P, 2, P], F32, tag="tp")
                for n, t in enumerate(chunks):
                    nc.tensor.transpose(tp[:, n, :], srcT[:, t * P : (t + 1) * P],
                                        ident[:])
                row = wk.tile([P, 2, P], dtype, tag=tag)
                nc.vector.tensor_copy(row[:], tp[:])
                return row

            def send_edges(edge_row):
                nc.sync.dma_start(
                    ag_in[:].rearrange("(n p) m -> p n m", p=P),
                    edge_row[:],
                )

            def do_ag(it, tag="halo"):
                ag_out = dram.tile([NCORES * 2 * P, H], BF16, tag=f"ago{it}",
                                   addr_space="Shared")
                nc.gpsimd.collective_compute(
                    "AllGather",
                    ALU.bypass,
                    replica_groups=[list(range(NCORES))],
                    ins=[ag_in[:].opt()],
                    outs=[ag_out[:].opt()],
                )
                agv = ag_out[:].rearrange("(n p) m -> p n m", p=P)
                halo = wk.tile([P, 2, P], BF16, tag=tag)
                nc.scalar.dma_start(halo[:, 0:1, :], agv[:, bass.ds(hl_v, 1), :])
                nc.sync.dma_start(halo[:, 1:2, :], agv[:, bass.ds(hr_v, 1), :])
                return halo

            # ---------- init: h row-form window (halo computed locally) ----------
            h_e = transpose_pair(hT, OWN_EDGE, "h_e")     # own edge rows of h
            h_i = transpose_pair(hT, OWN_INT, "h_i")      # own interior rows
            h_h = transpose_pair(hTh[:].rearrange("p a b -> p (a b)"), (0, 1),
                                 "h_h0", dtype=F32R)       # halo rows of h

            def to_r(src, tag):
                r = wk.tile([P, 2, P], F32R, tag=tag)
                nc.vector.tensor_copy(r[:], src[:])
                return r

            y_e, y_i = to_r(h_e, "y_e"), to_r(h_i, "y_i")
            y_T = hT_r

            # row-form accumulators for h window update
            acc_e = st.tile([P, 2, P], F32, tag="acc_e")
            acc_i = st.tile([P, 2, P], F32, tag="acc_i")
            acc_h = st.tile([P, 2, P], F32, tag="acc_h")
            accT = st.tile([P, R], F32, tag="accT")

            # ---------- RK4 loop: 16 ODE evaluations ----------
            k_h = None
            for it in range(16):
                sub = it % 4
                last = it == 15

                # ode_func: hn = S_own^T y_own + S_halo^T h_h (+ c*S_halo^T k_h)
                hn_ps = ps.tile([P, R], F32, tag="ps")
                for n, t in enumerate(s_own_order):
                    src = y_e if t in OWN_EDGE else y_i
                    idx = OWN_EDGE.index(t) if t in OWN_EDGE else OWN_INT.index(t)
                    nc.tensor.matmul(hn_ps[:], src[:, idx, :], s_own[t][:],
                                     start=(n == 0), stop=False)
                z1_ps = ps.tile([P, R], F32, tag="ps")
                nc.tensor.matmul(z1_ps[:], w1_r[:, 0, :], y_T[:],
                                 start=True, stop=False)
                for n in range(2):
                    nc.tensor.matmul(hn_ps[:], h_h[:, n, :], s_halo[n][:],
                                     start=False, stop=(sub == 0 and n == 1))
                if sub != 0:
                    sh = sh_full if sub == 3 else sh_half
                    with nc.allow_low_precision(reason="bf16 halo matmul"):
                        for n in range(2):
                            nc.tensor.matmul(hn_ps[:], k_h[:, n, :], sh[n][:],
                                             start=False, stop=(n == 1))
                hn_r = wk.tile([P, R], F32R, tag="hn_r")
                nc.scalar.activation(hn_r[:], hn_ps[:], AF.Copy, bias=0.0, scale=1.0)
                nc.tensor.matmul(z1_ps[:], w1_r[:, 1, :], hn_r[:],
                                 start=False, stop=True)
                th_r = wk.tile([P, R], F32R, tag="th_r")
                nc.scalar.activation(th_r[:], z1_ps[:], AF.Tanh, bias=b1_c[:],
                                     scale=1.0)
                # edge-first z2: edge columns of kt come off the critical path
                # to the collective sooner; interior follows.
                z2_ps4 = pst.tile([P, 4, P], F32, tag="z2e")
                nc.tensor.matmul(z2_ps4[:, 0, :], w2_r[:], th_r[:, 0:P])
                nc.tensor.matmul(z2_ps4[:, 1, :], w2_r[:], th_r[:, 3 * P : 4 * P])
                kt = wk.tile([P, R], F32, tag="kt", bufs=3)
                nc.vector.tensor_scalar(kt[:, 0:P], z2_ps4[:, 0, :], b2_c[:],
                                        None, ALU.add)
                nc.vector.tensor_scalar(kt[:, 3 * P : 4 * P], z2_ps4[:, 1, :],
                                        b2_c[:], None, ALU.add)

                def finish_kt():
                    z2i = z2_ps4[:, 2:4, :]
                    nc.tensor.matmul(z2i.rearrange("p a b -> p (a b)"),
                                     w2_r[:], th_r[:, P : 3 * P])
                    nc.scalar.activation(kt[:, P : 3 * P],
                                         z2i.rearrange("p a b -> p (a b)"),
                                         AF.Identity, bias=b2_c[:], scale=1.0)

                if last:
                    finish_kt()
                    acc4 = wk.tile([P, R], F32, tag="acc4")
                    nc.vector.tensor_tensor(acc4[:], accT[:], kt[:], ALU.add)
                    hT_fin = st.tile([P, R], F32, tag="hT_fin")
                    nc.vector.scalar_tensor_tensor(hT_fin[:], acc4[:], DT / 6.0,
                                                   hT[:], ALU.mult, ALU.add)
                    nc.sync.dma_start(out_d[:], hT_fin[:])
                    break

                # edge k rows -> collective (critical path): bf16 copy is
                # sent; a full-precision copy stays for local bookkeeping.
                tp_e = pst.tile([P, 2, P], F32, tag="tp")
                for n, t in enumerate(OWN_EDGE):
                    nc.tensor.transpose(tp_e[:, n, :], kt[:, t * P : (t + 1) * P],
                                        ident[:])
                k_e_bf = wk.tile([P, 2, P], BF16, tag="k_e_bf")
                with nc.allow_low_precision(reason="bf16 halo payload"):
                    nc.vector.tensor_copy(k_e_bf[:], tp_e[:])
                send_edges(k_e_bf)
                k_h = do_ag(it)
                k_e = wk.tile([P, 2, P], F32, tag="k_e")
                nc.vector.tensor_copy(k_e[:], tp_e[:])
                finish_kt()
                k_i = transpose_pair(kt, OWN_INT, "k_i")

                # T-form accumulator + next-y (overlap the collective)
                if sub == 0:
                    nc.vector.tensor_copy(accT[:], kt[:])
                elif sub in (1, 2):
                    nc.vector.scalar_tensor_tensor(accT[:], kt[:], 2.0, accT[:],
                                                   ALU.mult, ALU.add)

                if sub < 3:
                    coef = 0.5 * DT if sub < 2 else DT
                    y_T = wk.tile([P, R], F32R, tag="y_T", bufs=3)
                    nc.vector.scalar_tensor_tensor(y_T[:], kt[:], coef, hT[:],
                                                   ALU.mult, ALU.add)
                    # row-form next-y window
                    y_e = wk.tile([P, 2, P], F32R, tag="y_e")
                    nc.vector.scalar_tensor_tensor(y_e[:], k_e[:], coef, h_e[:],
                                                   ALU.mult, ALU.add)
                    y_i = wk.tile([P, 2, P], F32R, tag="y_i")
                    nc.vector.scalar_tensor_tensor(y_i[:], k_i[:], coef, h_i[:],
                                                   ALU.mult, ALU.add)
                    # row-form accumulators
                    if sub == 0:
                        for a, s in ((acc_e, k_e), (acc_i, k_i), (acc_h, k_h)):
                            nc.vector.tensor_copy(a[:], s[:])
                    else:
                        for a, s in ((acc_e, k_e), (acc_i, k_i), (acc_h, k_h)):
                            nc.vector.scalar_tensor_tensor(a[:], s[:], 2.0, a[:],
                                                           ALU.mult, ALU.add)
                else:
                    # step boundary: h' = h + dt/6 (acc + k4), rebuild windows
                    acc4 = wk.tile([P, R], F32, tag="acc4")
                    nc.vector.tensor_tensor(acc4[:], accT[:], kt[:], ALU.add)
                    hT_new = st.tile([P, R], F32, tag=f"hT{it}")
                    nc.vector.scalar_tensor_tensor(hT_new[:], acc4[:], DT / 6.0,
                                                   hT[:], ALU.mult, ALU.add)
                    hT = hT_new
                    hT_r = st.tile([P, R], F32R, tag=f"hTr{it}")
                    nc.vector.tensor_copy(hT_r[:], hT[:])
                    y_T = hT_r

                    new_h = []
                    for nm, a, s, h_old in (("e", acc_e, k_e, h_e),
                                            ("i", acc_i, k_i, h_i),
                                            ("h", acc_h, k_h, h_h)):
                        a4 = wk.tile([P, 2, P], F32, tag=f"a4{nm}")
                        nc.vector.tensor_tensor(a4[:], a[:], s[:], ALU.add)
                        hn_new = st.tile([P, 2, P], F32R if nm == "h" else F32,
                                         tag=f"h_{nm}{it}")
                        nc.vector.scalar_tensor_tensor(hn_new[:], a4[:], DT / 6.0,
                                                       h_old[:], ALU.mult, ALU.add)
                        new_h.append(hn_new)
                    h_e, h_i, h_h = new_h
                    y_e, y_i = to_r(h_e, "y_e"), to_r(h_i, "y_i")

    nc.compile()
    return nc


def get_nc():
    global _CACHED_NC
    if _CACHED_NC is None:
        _CACHED_NC = build_nc()
    return _CACHED_NC


def prep_inputs(features, speaker_ids, modality_masks, Wp, bp, W1, b1, W2, b2):
    features = np.asarray(features, dtype=np.float32)
    spk = np.asarray(speaker_ids).astype(np.float32)
    mm = np.asarray(modality_masks, dtype=np.float32)
    Wp = np.asarray(Wp, dtype=np.float32)
    bp = np.asarray(bp, dtype=np.float32)
    W1 = np.asarray(W1, dtype=np.float32)
    b1 = np.asarray(b1, dtype=np.float32)
    W2 = np.asarray(W2, dtype=np.float32)
    b2 = np.asarray(b2, dtype=np.float32)

    wp_pad = np.zeros((D_PAD, H), dtype=np.float32)
    wp_pad[:D_IN] = Wp
    s_all = mm.sum(axis=1)
    sv_all = (A2 - (A2 / 3.0) * s_all).astype(np.float32)
    svj_all = ((A2 / 3.0) * s_all).astype(np.float32)
    ident = np.eye(P, dtype=np.float32)

    def rep(v):
        return np.ascontiguousarray(np.broadcast_to(v, (P, v.shape[0])), dtype=np.float32)

    def pm(v):
        return np.ascontiguousarray(v.reshape(NW, P).T, dtype=np.float32)

    in_maps = []
    for c in range(NCORES):
        rows = slice(c * R, (c + 1) * R)
        rb = c * R - P
        jwin = np.arange(rb, rb + WIN)
        valid = (jwin >= 0) & (jwin < B)
        jcl = np.clip(jwin, 0, B - 1)
        jvals = np.where(valid, jwin.astype(np.float32), np.float32(SENT))
        xT = np.zeros((D_PAD, R), dtype=np.float32)
        xT[:D_IN] = features[rows].T
        xTh = np.zeros((D_PAD, 2 * P), dtype=np.float32)
        lo, hi = c * R - P, (c + 1) * R + P
        if lo >= 0:
            xTh[:D_IN, 0:P] = features[lo : c * R].T
        if hi <= B:
            xTh[:D_IN, P : 2 * P] = features[(c + 1) * R : hi].T
        ivals = np.arange(c * R, (c + 1) * R).astype(np.float32)
        hl_idx = 2 * (c - 1) + 1 if c > 0 else 0
        hr_idx = 2 * (c + 1) if c < NCORES - 1 else 0
        in_maps.append({
            "xT": xT,
            "xTh": xTh,
            "wp": wp_pad,
            "bp": bp.reshape(H, 1).copy(),
            "w1": W1.copy(),
            "b1": b1.reshape(H, 1).copy(),
            "w2": W2.copy(),
            "b2": b2.reshape(H, 1).copy(),
            "ident": ident,
            "iidx": rep(ivals),
            "spki": rep(spk[rows]),
            "ai": rep(mm[rows, 0]),
            "bi": rep(mm[rows, 1]),
            "ci": rep(mm[rows, 2]),
            "svi": rep(sv_all[rows]),
            "njw": pm(-jvals),
            "jw": pm(jvals),
            "spkj": pm(spk[jcl]),
            "aj": pm(mm[jcl, 0]),
            "bj": pm(mm[jcl, 1]),
            "cj": pm(mm[jcl, 2]),
            "svj": pm(svj_all[jcl]),
            "hidx": np.array([[hl_idx, hr_idx]], dtype=np.uint32),
        })
    return in_maps


def kernel(features, speaker_ids, modality_masks, Wp, bp, W1, b1, W2, b2,
           _runner=None):
    in_maps = prep_inputs(features, speaker_ids, modality_masks,
                          Wp, bp, W1, b1, W2, b2)
    nc = get_nc()
    if _runner is not None:
        results = _runner(nc, in_maps)
    else:
        results = run_bass_kernel_spmd(nc, in_maps, list(range(NCORES))).results
    out = np.concatenate([results[c]["hT_out"].T for c in range(NCORES)], axis=0)
    return np.ascontiguousarray(out, dtype=np.float32)



# revision 30
# speedup vs baseline: 1.3698x; 1.0299x over previous
"""Trainium2 Bass kernel for nn_DGODE (graph ODE over utterance nodes).

Self-contained: hardcodes all shapes. Strategy:
- Row-shard B=4096 nodes over 8 cores (512 rows each).
- The unnormalized adjacency S is symmetric and exp(-0.1|i-j|) decays so
  fast that entries with |i-j| > 128 are negligible relative to the row
  sum: each core builds only a banded window S[jwin, rows_c] (jwin = own
  rows +-128) directly in transposed orientation, with the row
  normalization folded in. It stays SBUF-resident for all 16 RK4 evals.
- Per ODE eval, the collective carries only each core's k = ode_func
  output edge rows (AllGather of [256,128]); each core maintains a
  row-form window of the RK4 base state h and assembles the next y
  window locally, so the banded matmul and first MLP matmul of the next
  eval overlap the collective.
- MLP entirely in transposed orientation so every matmul has N=512;
  matmuls in float32r (full-rate fp32, ~1e-4 precision).

Row-form window layout (chunks of 128 rows): own rows stored as
[edge0 (rows 0:128), edge1 (rows 384:512)] + [int0 (128:256), int1
(256:384)] so the collective input (the two edges) is ready after only
two PE transposes.
"""

import sys

if "/opt/trn_rl_repo" not in sys.path:
    sys.path.insert(0, "/opt/trn_rl_repo")

import numpy as np

import concourse.bacc as bacc
import concourse.bass as bass
import concourse.mybir as mybir
import concourse.tile as tile
from concourse.bass_utils import run_bass_kernel_spmd

F32 = mybir.dt.float32
F32R = mybir.dt.float32r
BF16 = mybir.dt.bfloat16
U32 = mybir.dt.uint32
AF = mybir.ActivationFunctionType
ALU = mybir.AluOpType

NCORES = 8
B = 4096
D_IN = 1856
D_PAD = 1920           # 15 * 128
ND = D_PAD // 128
H = 128
R = B // NCORES        # 512 rows per core
P = 128
NW = 6                 # window chunks: [halo_l | own x4 | halo_r]
WIN = NW * P           # 768-row banded window (W = 128)
N_STEPS = 4
DT = 1.0 / N_STEPS
A1, A2, BETA = 0.8, 0.5, 0.1
SENT = -3.0e7          # sentinel j for out-of-range window rows -> S = 0

# own-chunk storage order [e0, i0, i1, e1] -> window chunk index 1 + t
OWN_EDGE = (0, 3)      # window own chunks held in the "edge" tiles
OWN_INT = (1, 2)

_CACHED_NC = None


def build_nc():
    nc = bacc.Bacc(
        "TRN2",
        target_bir_lowering=False,
        debug=False,
        enable_asserts=True,
        num_devices=NCORES,
    )

    # ---- per-core external inputs ----
    xT_d = nc.dram_tensor("xT", [D_PAD, R], F32R, kind="ExternalInput")
    xTh_d = nc.dram_tensor("xTh", [D_PAD, 2 * P], F32R, kind="ExternalInput")
    wp_d = nc.dram_tensor("wp", [D_PAD, H], F32R, kind="ExternalInput")
    bp_d = nc.dram_tensor("bp", [H, 1], F32, kind="ExternalInput")
    w1_d = nc.dram_tensor("w1", [2 * H, H], F32R, kind="ExternalInput")
    b1_d = nc.dram_tensor("b1", [H, 1], F32, kind="ExternalInput")
    w2_d = nc.dram_tensor("w2", [H, H], F32R, kind="ExternalInput")
    b2_d = nc.dram_tensor("b2", [H, 1], F32, kind="ExternalInput")
    ident_d = nc.dram_tensor("ident", [P, P], F32, kind="ExternalInput")
    iidx_d = nc.dram_tensor("iidx", [P, R], F32, kind="ExternalInput")
    spki_d = nc.dram_tensor("spki", [P, R], F32, kind="ExternalInput")
    ai_d = nc.dram_tensor("ai", [P, R], F32, kind="ExternalInput")
    bi_d = nc.dram_tensor("bi", [P, R], F32, kind="ExternalInput")
    ci_d = nc.dram_tensor("ci", [P, R], F32, kind="ExternalInput")
    svi_d = nc.dram_tensor("svi", [P, R], F32, kind="ExternalInput")
    njw_d = nc.dram_tensor("njw", [P, NW], F32, kind="ExternalInput")
    jw_d = nc.dram_tensor("jw", [P, NW], F32, kind="ExternalInput")
    spkj_d = nc.dram_tensor("spkj", [P, NW], F32, kind="ExternalInput")
    aj_d = nc.dram_tensor("aj", [P, NW], F32, kind="ExternalInput")
    bj_d = nc.dram_tensor("bj", [P, NW], F32, kind="ExternalInput")
    cj_d = nc.dram_tensor("cj", [P, NW], F32, kind="ExternalInput")
    svj_d = nc.dram_tensor("svj", [P, NW], F32, kind="ExternalInput")
    hidx_d = nc.dram_tensor("hidx", [1, 2], U32, kind="ExternalInput")

    out_d = nc.dram_tensor("hT_out", [H, R], F32, kind="ExternalOutput")

    with tile.TileContext(nc) as tc:
        with (
            tc.tile_pool(name="consts", bufs=1) as cs,
            tc.tile_pool(name="work", bufs=2) as wk,
            tc.tile_pool(name="states", bufs=1) as st,
            tc.tile_pool(name="ps", bufs=3, space="PSUM") as ps,
            tc.tile_pool(name="pst", bufs=2, space="PSUM") as pst,
            tc.tile_pool(name="dram", bufs=1, space="DRAM") as dram,
        ):
            # ---------- collective warm-up (overlaps the setup phase) ----------
            warm_f = cs.tile([1, P], F32, tag="warm")
            nc.vector.memset(warm_f[:], 0.0)
            warm_in = dram.tile([1, P], F32, tag="warm_in")
            warm_out = dram.tile([NCORES, P], F32, tag="warm_out",
                                 addr_space="Shared")
            nc.gpsimd.dma_start(warm_in[:], warm_f[:])
            nc.gpsimd.collective_compute(
                "AllGather",
                ALU.bypass,
                replica_groups=[list(range(NCORES))],
                ins=[warm_in[:].opt()],
                outs=[warm_out[:].opt()],
            )

            # ---------- load constants ----------
            def load_const(dram_t, shape, name, rdtype=None):
                t = cs.tile(shape, F32, tag=name)
                nc.sync.dma_start(t[:], dram_t[:])
                if rdtype is None:
                    return t
                tr = cs.tile(shape, rdtype, tag=name + "_r")
                nc.vector.tensor_copy(tr[:], t[:])
                return tr

            wp_r = cs.tile([P, ND, H], F32R, tag="wp_r")
            nc.sync.dma_start(wp_r[:], wp_d[:].rearrange("(n p) m -> p n m", p=P))
            w1_r = cs.tile([P, 2, H], F32R, tag="w1_r")
            nc.sync.dma_start(w1_r[:], w1_d[:].rearrange("(n p) m -> p n m", p=P))
            w2_r = cs.tile([H, H], F32R, tag="w2_r")
            nc.sync.dma_start(w2_r[:], w2_d[:])
            ident = load_const(ident_d, [P, P], "ident")
            bp_c = load_const(bp_d, [H, 1], "bp")
            b1_c = load_const(b1_d, [H, 1], "b1")
            b2_c = load_const(b2_d, [H, 1], "b2")

            iidx = load_const(iidx_d, [P, R], "iidx")
            spki = load_const(spki_d, [P, R], "spki")
            ai = load_const(ai_d, [P, R], "ai")
            bi = load_const(bi_d, [P, R], "bi")
            ci = load_const(ci_d, [P, R], "ci")
            svi = load_const(svi_d, [P, R], "svi")
            njw = load_const(njw_d, [P, NW], "njw")
            jw = load_const(jw_d, [P, NW], "jw")
            spkj = load_const(spkj_d, [P, NW], "spkj")
            aj = load_const(aj_d, [P, NW], "aj")
            bj = load_const(bj_d, [P, NW], "bj")
            cj = load_const(cj_d, [P, NW], "cj")
            svj = load_const(svj_d, [P, NW], "svj")

            hidx_sb = cs.tile([1, 2], U32, tag="hidx")
            nc.sync.dma_start(hidx_sb[:], hidx_d[:])
            regs_l = nc.alloc_registers("hl_reg")
            nc.regs_load(regs_l, hidx_sb[0:1, 0:1])
            hl_v = nc.snap(regs_l, donate=True)
            regs_r = nc.alloc_registers("hr_reg")
            nc.regs_load(regs_r, hidx_sb[0:1, 1:2])
            hr_v = nc.snap(regs_r, donate=True)

            # ---------- input projection: hT = (X @ Wp).T + bp ----------
            # halo columns projected locally too (features for own rows
            # +-128 are fed per-core), so no init collective is needed.
            with tc.tile_pool(name="xtp", bufs=3) as xtp:
                xv = xT_d[:].rearrange("(n p) m -> p n m", p=P)
                xhv = xTh_d[:].rearrange("(n p) m -> p n m", p=P)
                h0_ps = ps.tile([P, R], F32, tag="ps")
                h0h_ps4 = pst.tile([P, 4, P], F32, tag="z2e")
                h0h_ps = h0h_ps4[:, 0:2, :]
                for d in range(ND):
                    xd = xtp.tile([P, R], F32R, tag="xd")
                    nc.sync.dma_start(xd[:], xv[:, d, :])
                    nc.tensor.matmul(
                        h0_ps[:], wp_r[:, d, :], xd[:],
                        start=(d == 0), stop=(d == ND - 1),
                    )
                    xhd = xtp.tile([P, 2 * P], F32R, tag="xhd")
                    nc.scalar.dma_start(xhd[:], xhv[:, d, :])
                    nc.tensor.matmul(
                        h0h_ps.rearrange("p a b -> p (a b)"),
                        wp_r[:, d, :], xhd[:],
                        start=(d == 0), stop=(d == ND - 1),
                    )
            hT = st.tile([P, R], F32, tag="hT")
            nc.scalar.activation(hT[:], h0_ps[:], AF.Identity, bias=bp_c[:], scale=1.0)
            hT_r = st.tile([P, R], F32R, tag="hT_r")
            nc.scalar.activation(hT_r[:], h0_ps[:], AF.Identity, bias=bp_c[:], scale=1.0)
            hTh = st.tile([P, 2, P], F32, tag="hTh")
            nc.scalar.activation(hTh[:], h0h_ps, AF.Identity, bias=bp_c[:],
                                 scale=1.0)

            # ---------- banded adjacency build (transposed, unnormalized) ----------
            s_tiles = []
            for k in range(NW):
                adt = wk.tile([P, R], F32, tag="adt")
                nc.scalar.activation(adt[:], iidx[:], AF.Abs,
                                     bias=njw[:, k : k + 1], scale=1.0)
                Tt = wk.tile([P, R], F32, tag="Tt")
                nc.scalar.activation(Tt[:], adt[:], AF.Exp, scale=-BETA)

                Pm = wk.tile([P, R], F32, tag="Pm")
                nc.vector.tensor_scalar(Pm[:], spki[:], spkj[:, k : k + 1], None,
                                        ALU.is_equal)
                m1 = wk.tile([P, R], F32, tag="m1")
                nc.vector.tensor_scalar(m1[:], ai[:], aj[:, k : k + 1], None, ALU.min)
                m2 = wk.tile([P, R], F32, tag="m2")
                nc.vector.scalar_tensor_tensor(m2[:], bi[:], bj[:, k : k + 1], m1[:],
                                               ALU.min, ALU.add)
                m3 = wk.tile([P, R], F32, tag="m3")
                nc.vector.scalar_tensor_tensor(m3[:], ci[:], cj[:, k : k + 1], m2[:],
                                               ALU.min, ALU.add)
                u0 = wk.tile([P, R], F32, tag="u0")
                nc.vector.scalar_tensor_tensor(u0[:], m3[:], 2.0 * A2 / 3.0, svi[:],
                                               ALU.mult, ALU.add)
                uu = wk.tile([P, R], F32, tag="uu")
                nc.vector.tensor_scalar(uu[:], u0[:], svj[:, k : k + 1], None,
                                        ALU.subtract)
                t1 = wk.tile([P, R], F32, tag="t1")
                nc.vector.tensor_scalar(t1[:], uu[:], -1.0, A1, ALU.mult, ALU.add)
                t2 = wk.tile([P, R], F32, tag="t2")
                nc.gpsimd.tensor_tensor(t2[:], Pm[:], t1[:], ALU.mult)
                qq = wk.tile([P, R], F32, tag="qq")
                nc.vector.tensor_tensor(qq[:], uu[:], t2[:], ALU.add)
                s0 = wk.tile([P, R], F32, tag="s0")
                nc.vector.tensor_tensor(s0[:], Tt[:], qq[:], ALU.mult)
                dm = wk.tile([P, R], F32, tag="dm")
                nc.vector.tensor_scalar(dm[:], iidx[:], jw[:, k : k + 1], 1.0 - A1,
                                        ALU.is_equal, ALU.mult)
                sk = cs.tile([P, R], F32R, tag=f"sk{k}")
                nc.vector.tensor_tensor(sk[:], s0[:], dm[:], ALU.add)
                s_tiles.append(sk)

            # ---------- row sums d_i, reciprocal, fold into S ----------
            ones_f = cs.tile([P, 1], F32, tag="ones_f")
            nc.vector.memset(ones_f[:], 1.0)
            ones_r = cs.tile([P, 1], F32R, tag="ones")
            nc.vector.tensor_copy(ones_r[:], ones_f[:])
            d_ps = ps.tile([P, R], F32, tag="ps")
            for k in range(NW):
                nc.tensor.matmul(d_ps[0:1, :], ones_r[:], s_tiles[k][:],
                                 start=(k == 0), stop=(k == NW - 1))
            dsum = cs.tile([1, R], F32, tag="dsum")
            nc.vector.tensor_scalar(dsum[:], d_ps[0:1, :], 1e-8, None, ALU.add)
            rd = cs.tile([1, R], F32R, tag="rd")
            with nc.allow_low_precision(reason="f32r is full-width storage"):
                nc.vector.reciprocal(rd[:], dsum[:])
            onesrow_f = cs.tile([1, P], F32, tag="onesrow_f")
            nc.vector.memset(onesrow_f[:], 1.0)
            onesrow_r = cs.tile([1, P], F32R, tag="onesrow")
            nc.vector.tensor_copy(onesrow_r[:], onesrow_f[:])
            b2row_f = cs.tile([1, H], F32, tag="b2row_f")
            nc.sync.dma_start(b2row_f[:], b2_d[:].rearrange("h o -> o h"))
            b2row = cs.tile([1, H], F32R, tag="b2row")
            nc.vector.tensor_copy(b2row[:], b2row_f[:])
            rdb_ps = ps.tile([P, R], F32, tag="ps")
            nc.tensor.matmul(rdb_ps[:], onesrow_r[:], rd[:])
            for k in range(NW):
                nc.vector.tensor_tensor(s_tiles[k][:], s_tiles[k][:], rdb_ps[:],
                                        ALU.mult)

            # s_tiles for the own chunks in [e0, i0, i1, e1] order, halos:
            s_own = [s_tiles[1], s_tiles[2], s_tiles[3], s_tiles[4]]
            s_own_order = [0, 3, 1, 2]      # issue edges' MMs first
            s_halo = [s_tiles[0], s_tiles[5]]
            # RK4-coefficient-prescaled halo S so the post-fetch halo matmuls
            # consume the raw gathered k directly (no y_h build on the path).
            # bf16: the exchanged halo k is low-stakes (decayed S weights),
            # so the collective payload and its S multiplier run in bf16.
            sh_half, sh_full = [], []
            with nc.allow_low_precision(reason="bf16 halo-exchange payload"):
                for n in range(2):
                    t_h = cs.tile([P, R], BF16, tag=f"shh{n}")
                    nc.vector.tensor_scalar(t_h[:], s_halo[n][:], 0.5 * DT,
                                            None, ALU.mult)
                    sh_half.append(t_h)
                    t_f = cs.tile([P, R], BF16, tag=f"shf{n}")
                    nc.vector.tensor_scalar(t_f[:], s_halo[n][:], DT, None,
                                            ALU.mult)
                    sh_full.append(t_f)

            ag_in = dram.tile([2 * P, H], BF16, tag="ag_in")

            def transpose_pair(srcT, chunks, tag, dtype=F32):
                """Transpose two [128,128] column blocks of a T-form tile
                into a row-form [P,2,P] tile."""
                tp = pst.tile([P, 2, P], F32, tag="tp")
                for n, t in enumerate(chunks):
                    nc.tensor.transpose(tp[:, n, :], srcT[:, t * P : (t + 1) * P],
                                        ident[:])
                row = wk.tile([P, 2, P], dtype, tag=tag)
                nc.vector.tensor_copy(row[:], tp[:])
                return row

            def send_edges(edge_row):
                nc.sync.dma_start(
                    ag_in[:].rearrange("(n p) m -> p n m", p=P),
                    edge_row[:],
                )

            def do_ag(it, tag="halo"):
                ag_out = dram.tile([NCORES * 2 * P, H], BF16, tag=f"ago{it}",
                                   addr_space="Shared")
                nc.gpsimd.collective_compute(
                    "AllGather",
                    ALU.bypass,
                    replica_groups=[list(range(NCORES))],
                    ins=[ag_in[:].opt()],
                    outs=[ag_out[:].opt()],
                )
                agv = ag_out[:].rearrange("(n p) m -> p n m", p=P)
                halo = wk.tile([P, 2, P], BF16, tag=tag)
                nc.scalar.dma_start(halo[:, 0:1, :], agv[:, bass.ds(hl_v, 1), :])
                nc.sync.dma_start(halo[:, 1:2, :], agv[:, bass.ds(hr_v, 1), :])
                return halo

            # ---------- init: h row-form window (halo computed locally) ----------
            h_e = transpose_pair(hT, OWN_EDGE, "h_e")     # own edge rows of h
            h_i = transpose_pair(hT, OWN_INT, "h_i")      # own interior rows
            h_h = transpose_pair(hTh[:].rearrange("p a b -> p (a b)"), (0, 1),
                                 "h_h0", dtype=F32R)       # halo rows of h

            def to_r(src, tag):
                r = wk.tile([P, 2, P], F32R, tag=tag)
                nc.vector.tensor_copy(r[:], src[:])
                return r

            y_e, y_i = to_r(h_e, "y_e"), to_r(h_i, "y_i")
            y_T = hT_r

            # row-form accumulators for h window update
            acc_e = st.tile([P, 2, P], F32, tag="acc_e")
            acc_i = st.tile([P, 2, P], F32, tag="acc_i")
            acc_h = st.tile([P, 2, P], F32, tag="acc_h")
            accT = st.tile([P, R], F32, tag="accT")

            # ---------- RK4 loop: 16 ODE evaluations ----------
            k_h = None
            for it in range(16):
                sub = it % 4
                last = it == 15

                # ode_func: hn = S_own^T y_own + S_halo^T h_h (+ c*S_halo^T k_h)
                hn_ps = ps.tile([P, R], F32, tag="ps")
                for n, t in enumerate(s_own_order):
                    src = y_e if t in OWN_EDGE else y_i
                    idx = OWN_EDGE.index(t) if t in OWN_EDGE else OWN_INT.index(t)
                    nc.tensor.matmul(hn_ps[:], src[:, idx, :], s_own[t][:],
                                     start=(n == 0), stop=False)
                z1_ps = ps.tile([P, R], F32, tag="ps")
                nc.tensor.matmul(z1_ps[:], w1_r[:, 0, :], y_T[:],
                                 start=True, stop=False)
                for n in range(2):
                    nc.tensor.matmul(hn_ps[:], h_h[:, n, :], s_halo[n][:],
                                     start=False, stop=(sub == 0 and n == 1))
                if sub != 0:
                    sh = sh_full if sub == 3 else sh_half
                    with nc.allow_low_precision(reason="bf16 halo matmul"):
                        for n in range(2):
                            nc.tensor.matmul(hn_ps[:], k_h[:, n, :], sh[n][:],
                                             start=False, stop=(n == 1))
                hn_r = wk.tile([P, R], F32R, tag="hn_r")
                nc.scalar.activation(hn_r[:], hn_ps[:], AF.Copy, bias=0.0, scale=1.0)
                nc.tensor.matmul(z1_ps[:], w1_r[:, 1, :], hn_r[:],
                                 start=False, stop=True)
                th_r = wk.tile([P, R], F32R, tag="th_r")
                nc.scalar.activation(th_r[:], z1_ps[:], AF.Tanh, bias=b1_c[:],
                                     scale=1.0)
                # k edges directly in ROW form: th_edge^T @ W2 + 1^T b2row.
                # The T-form z2/kt (local bookkeeping) moves off the send path.
                kt = wk.tile([P, R], F32, tag="kt", bufs=3)
                z2_ps4 = pst.tile([P, 4, P], F32, tag="z2e")

                def finish_kt():
                    nc.tensor.matmul(z2_ps4[:, 0, :], w2_r[:], th_r[:, 0:P])
                    nc.tensor.matmul(z2_ps4[:, 1, :], w2_r[:],
                                     th_r[:, 3 * P : 4 * P])
                    nc.vector.tensor_scalar(kt[:, 0:P], z2_ps4[:, 0, :],
                                            b2_c[:], None, ALU.add)
                    nc.vector.tensor_scalar(kt[:, 3 * P : 4 * P],
                                            z2_ps4[:, 1, :], b2_c[:], None,
                                            ALU.add)
                    z2i = z2_ps4[:, 2:4, :]
                    nc.tensor.matmul(z2i.rearrange("p a b -> p (a b)"),
                                     w2_r[:], th_r[:, P : 3 * P])
                    nc.scalar.activation(kt[:, P : 3 * P],
                                         z2i.rearrange("p a b -> p (a b)"),
                                         AF.Identity, bias=b2_c[:], scale=1.0)

                if last:
                    finish_kt()
                    acc4 = wk.tile([P, R], F32, tag="acc4")
                    nc.vector.tensor_tensor(acc4[:], accT[:], kt[:], ALU.add)
                    hT_fin = st.tile([P, R], F32, tag="hT_fin")
                    nc.vector.scalar_tensor_tensor(hT_fin[:], acc4[:], DT / 6.0,
                                                   hT[:], ALU.mult, ALU.add)
                    nc.sync.dma_start(out_d[:], hT_fin[:])
                    break

                # edge k rows -> collective (critical path): bf16 copy is
                # sent; a full-precision copy stays for local bookkeeping.
                tp_e = pst.tile([P, 2, P], F32, tag="tp")
                for n, t in enumerate(OWN_EDGE):
                    nc.tensor.matmul(tp_e[:, n, :], th_r[:, t * P : (t + 1) * P],
                                     w2_r[:], start=True, stop=False)
                    nc.tensor.matmul(tp_e[:, n, :], onesrow_r[:], b2row[:],
                                     start=False, stop=True)
                k_e_bf = wk.tile([P, 2, P], BF16, tag="k_e_bf")
                with nc.allow_low_precision(reason="bf16 halo payload"):
                    nc.vector.tensor_copy(k_e_bf[:], tp_e[:])
                send_edges(k_e_bf)
                k_h = do_ag(it)
                k_e = wk.tile([P, 2, P], F32, tag="k_e")
                nc.vector.tensor_copy(k_e[:], tp_e[:])
                finish_kt()
                k_i = transpose_pair(kt, OWN_INT, "k_i")

                # T-form accumulator + next-y (overlap the collective)
                if sub == 0:
                    nc.vector.tensor_copy(accT[:], kt[:])
                elif sub in (1, 2):
                    nc.vector.scalar_tensor_tensor(accT[:], kt[:], 2.0, accT[:],
                                                   ALU.mult, ALU.add)

                if sub < 3:
                    coef = 0.5 * DT if sub < 2 else DT
                    y_T = wk.tile([P, R], F32R, tag="y_T", bufs=3)
                    nc.vector.scalar_tensor_tensor(y_T[:], kt[:], coef, hT[:],
                                                   ALU.mult, ALU.add)
                    # row-form next-y window
                    y_e = wk.tile([P, 2, P], F32R, tag="y_e")
                    nc.vector.scalar_tensor_tensor(y_e[:], k_e[:], coef, h_e[:],
                                                   ALU.mult, ALU.add)
                    y_i = wk.tile([P, 2, P], F32R, tag="y_i")
                    nc.vector.scalar_tensor_tensor(y_i[:], k_i[:], coef, h_i[:],
                                                   ALU.mult, ALU.add)
                    # row-form accumulators
                    if sub == 0:
                        for a, s in ((acc_e, k_e), (acc_i, k_i), (acc_h, k_h)):
                            nc.vector.tensor_copy(a[:], s[:])
                    else:
                        for a, s in ((acc_e, k_e), (acc_i, k_i), (acc_h, k_h)):
                            nc.vector.scalar_tensor_tensor(a[:], s[:], 2.0, a[:],
                                                           ALU.mult, ALU.add)
                else:
                    # step boundary: h' = h + dt/6 (acc + k4), rebuild windows
                    acc4 = wk.tile([P, R], F32, tag="acc4")
                    nc.vector.tensor_tensor(acc4[:], accT[:], kt[:], ALU.add)
                    hT_new = st.tile([P, R], F32, tag=f"hT{it}")
                    nc.vector.scalar_tensor_tensor(hT_new[:], acc4[:], DT / 6.0,
                                                   hT[:], ALU.mult, ALU.add)
                    hT = hT_new
                    hT_r = st.tile([P, R], F32R, tag=f"hTr{it}")
                    nc.vector.tensor_copy(hT_r[:], hT[:])
                    y_T = hT_r

                    new_h = []
                    for nm, a, s, h_old in (("e", acc_e, k_e, h_e),
                                            ("i", acc_i, k_i, h_i),
                                            ("h", acc_h, k_h, h_h)):
                        a4 = wk.tile([P, 2, P], F32, tag=f"a4{nm}")
                        nc.vector.tensor_tensor(a4[:], a[:], s[:], ALU.add)
                        hn_new = st.tile([P, 2, P], F32R if nm == "h" else F32,
                                         tag=f"h_{nm}{it}")
                        nc.vector.scalar_tensor_tensor(hn_new[:], a4[:], DT / 6.0,
                                                       h_old[:], ALU.mult, ALU.add)
                        new_h.append(hn_new)
                    h_e, h_i, h_h = new_h
                    y_e, y_i = to_r(h_e, "y_e"), to_r(h_i, "y_i")

    nc.compile()
    return nc


def get_nc():
    global _CACHED_NC
    if _CACHED_NC is None:
        _CACHED_NC = build_nc()
    return _CACHED_NC


def prep_inputs(features, speaker_ids, modality_masks, Wp, bp, W1, b1, W2, b2):
    features = np.asarray(features, dtype=np.float32)
    spk = np.asarray(speaker_ids).astype(np.float32)
    mm = np.asarray(modality_masks, dtype=np.float32)
    Wp = np.asarray(Wp, dtype=np.float32)
    bp = np.asarray(bp, dtype=np.float32)
    W1 = np.asarray(W1, dtype=np.float32)
    b1 = np.asarray(b1, dtype=np.float32)
    W2 = np.asarray(W2, dtype=np.float32)
    b2 = np.asarray(b2, dtype=np.float32)

    wp_pad = np.zeros((D_PAD, H), dtype=np.float32)
    wp_pad[:D_IN] = Wp
    s_all = mm.sum(axis=1)
    sv_all = (A2 - (A2 / 3.0) * s_all).astype(np.float32)
    svj_all = ((A2 / 3.0) * s_all).astype(np.float32)
    ident = np.eye(P, dtype=np.float32)

    def rep(v):
        return np.ascontiguousarray(np.broadcast_to(v, (P, v.shape[0])), dtype=np.float32)

    def pm(v):
        return np.ascontiguousarray(v.reshape(NW, P).T, dtype=np.float32)

    in_maps = []
    for c in range(NCORES):
        rows = slice(c * R, (c + 1) * R)
        rb = c * R - P
        jwin = np.arange(rb, rb + WIN)
        valid = (jwin >= 0) & (jwin < B)
        jcl = np.clip(jwin, 0, B - 1)
        jvals = np.where(valid, jwin.astype(np.float32), np.float32(SENT))
        xT = np.zeros((D_PAD, R), dtype=np.float32)
        xT[:D_IN] = features[rows].T
        xTh = np.zeros((D_PAD, 2 * P), dtype=np.float32)
        lo, hi = c * R - P, (c + 1) * R + P
        if lo >= 0:
            xTh[:D_IN, 0:P] = features[lo : c * R].T
        if hi <= B:
            xTh[:D_IN, P : 2 * P] = features[(c + 1) * R : hi].T
        ivals = np.arange(c * R, (c + 1) * R).astype(np.float32)
        hl_idx = 2 * (c - 1) + 1 if c > 0 else 0
        hr_idx = 2 * (c + 1) if c < NCORES - 1 else 0
        in_maps.append({
            "xT": xT,
            "xTh": xTh,
            "wp": wp_pad,
            "bp": bp.reshape(H, 1).copy(),
            "w1": W1.copy(),
            "b1": b1.reshape(H, 1).copy(),
            "w2": W2.copy(),
            "b2": b2.reshape(H, 1).copy(),
            "ident": ident,
            "iidx": rep(ivals),
            "spki": rep(spk[rows]),
            "ai": rep(mm[rows, 0]),
            "bi": rep(mm[rows, 1]),
            "ci": rep(mm[rows, 2]),
            "svi": rep(sv_all[rows]),
            "njw": pm(-jvals),
            "jw": pm(jvals),
            "spkj": pm(spk[jcl]),
            "aj": pm(mm[jcl, 0]),
            "bj": pm(mm[jcl, 1]),
            "cj": pm(mm[jcl, 2]),
            "svj": pm(svj_all[jcl]),
            "hidx": np.array([[hl_idx, hr_idx]], dtype=np.uint32),
        })
    return in_maps


def kernel(features, speaker_ids, modality_masks, Wp, bp, W1, b1, W2, b2,
           _runner=None):
    in_maps = prep_inputs(features, speaker_ids, modality_masks,
                          Wp, bp, W1, b1, W2, b2)
    nc = get_nc()
    if _runner is not None:
        results = _runner(nc, in_maps)
    else:
        results = run_bass_kernel_spmd(nc, in_maps, list(range(NCORES))).results
    out = np.concatenate([results[c]["hT_out"].T for c in range(NCORES)], axis=0)
    return np.ascontiguousarray(out, dtype=np.float32)

